# revision 6
# baseline (speedup 1.0000x reference)
"""TRN2 Bass kernel for nn_BilinearInteraction.

Math: out[b,k] = sum_{e,f} E[b,i(k),e] * W[k,e,f] * E[b,j(k),f]
for the 780 upper-triangular field pairs (i<j) of 40 fields, embed dim 32.

Strategy (per core, batch-sharded 8 ways, B_CORE=1024):
- Host packs transposed embeddings ET[128=(g%4)*32+e, g//4, b] (f32 and f16
  copies) and W into "quads": quad (i, m) covers pairs (i, 4m+c), c=0..3
  (j-block m is 4-field aligned so the fp16 multiplier is one tile slice).
- Stage 1 (PE, float32r @ 1 cyc/row): u[(c,f), b] = Wq.T @ ET_i  (128x512)
- Evict PSUM->SBUF fp16 (ScalarE) then multiply by ET16 block m (VectorE,
  2x mode); every 4th quad multiplies straight from PSUM (fp32, 1x) to
  offload ScalarE.
- Stage 2 (PE, fp16): indicator matmul folds f-groups: out2[4g+c, b] +=
  sum_f t[(c,f), b]; 32 quads accumulate into one PSUM tile.
- Evict out2 to SBUF, DMA to DRAM; host gathers slots -> (8192, 780).
"""

import numpy as np

import concourse.bass as bass
import concourse.mybir as mybir
import concourse.tile as tile
from concourse import bacc
from concourse.bass_utils import run_bass_kernel_spmd

# ---------------- problem constants (hardcoded) ----------------
NF = 40          # fields
E = 32           # embed dim
NPAIR = 780      # NF*(NF-1)/2
NB = NF // 4     # 10 aligned j-blocks
BATCH = 8192
NCORES = 8
B_CORE = BATCH // NCORES   # 1024
CHUNK = 512
NCHUNK = B_CORE // CHUNK   # 2
PATH_B_EVERY = 4           # every 4th quad: DVE multiplies direct from PSUM

# ---------------- quad tables (pure python, deterministic) ----------------
_quads = []
for _i in range(NF):
    for _m in range(NB):
        if 4 * _m + 3 > _i:          # block m has some j > i
            _quads.append((_i, _m))
NQ = len(_quads)                      # 210
NT = (4 * NQ + 127) // 128            # 7 out2 tiles of 128 slots

_pair2k = {}
_k = 0
for _i in range(NF):
    for _j in range(_i + 1, NF):
        _pair2k[(_i, _j)] = _k
        _k += 1

# Matmul operand base partitions must be in {0, 32, 64}. Residues 0-2 use
# K=32 at base 32*r and share W columns 0..54; residue 3 uses K=64 at base
# 64 (rows 64:96 zeroed, W at 96:128) in its own column range 55..99.
_res_count = [0, 0, 0, 0]
QUAD_META = []                        # (i, m, r, idx, kbase, ksize)
for _i, _m in _quads:
    _r = _i % 4
    _idx = _res_count[_r]
    _res_count[_r] += 1
    if _r < 3:
        QUAD_META.append((_i, _m, _r, _idx, 32 * _r, 32))
    else:
        QUAD_META.append((_i, _m, _r, 55 + _idx, 64, 64))
WL = max(_res_count[:3]) + _res_count[3]   # 55 + 45 = 100

SLOT_OF_K = np.full(NPAIR, -1, np.int64)
for _q, (_i, _m) in enumerate(_quads):
    for _c in range(4):
        _j = 4 * _m + _c
        if _j > _i:
            SLOT_OF_K[_pair2k[(_i, _j)]] = 4 * _q + _c
assert (SLOT_OF_K >= 0).all()


# ---------------- host packing ----------------
def _pack_w(W):
    Wp = np.zeros((128, WL, 128), np.float32)
    for (i, m, r, idx, kbase, ksize) in QUAD_META:
        for c in range(4):
            j = 4 * m + c
            if j > i:
                Wp[32 * r:32 * r + 32, idx, 32 * c:32 * c + 32] = W[_pair2k[(i, j)]]
    return Wp


def _pack_et(emb):
    # emb (8192, 40, 32) -> (8 cores, 128, 10, 1024); partition = (g%4)*32+e
    et = emb.reshape(NCORES, B_CORE, NB, 4, E).transpose(0, 3, 4, 2, 1)
    return np.ascontiguousarray(et.reshape(NCORES, 128, NB, B_CORE))


def _make_ind():
    ind = np.zeros((128, 32, 128), np.float16)
    for g in range(32):
        for c in range(4):
            for f in range(E):
                ind[32 * c + f, g, 4 * g + c] = 1.0
    return ind


# ---------------- bass program ----------------
_CACHED = None


def _build():
    global _CACHED
    if _CACHED is not None:
        return _CACHED

    nc = bacc.Bacc("TRN2", target_bir_lowering=False, debug=False)
    f32r = mybir.dt.float32r
    f32 = mybir.dt.float32
    f16 = mybir.dt.float16

    et32_d = nc.dram_tensor("et32", [128, NB, B_CORE], f32r, kind="ExternalInput")
    et16_d = nc.dram_tensor("et16", [128, NB, B_CORE], f16, kind="ExternalInput")
    wp_d = nc.dram_tensor("wp", [128, WL, 128], f32r, kind="ExternalInput")
    ind_d = nc.dram_tensor("ind", [128, 32, 128], f16, kind="ExternalInput")
    o_d = nc.dram_tensor("o", [NT, 128, NCHUNK, CHUNK], f32, kind="ExternalOutput")

    with tile.TileContext(nc) as tc:
        with (
            tc.tile_pool(name="consts", bufs=1) as consts,
            tc.tile_pool(name="work", bufs=4) as work,
            tc.tile_pool(name="outs", bufs=4) as outs,
            tc.tile_pool(name="upsum", bufs=4, space="PSUM") as upsum,
            tc.tile_pool(name="opsum", bufs=2, space="PSUM") as opsum,
        ):
            wp_sb = consts.tile([128, WL, 128], f32r)
            nc.sync.dma_start(out=wp_sb[:], in_=wp_d[:])
            ind_sb = consts.tile([128, 32, 128], f16)
            nc.sync.dma_start(out=ind_sb[:], in_=ind_d[:])
            et32_sb = consts.tile([128, NB, B_CORE], f32r)
            et16_sb = consts.tile([128, NB, B_CORE], f16)
            for m in range(NB):
                nc.sync.dma_start(out=et32_sb[:, m, :], in_=et32_d[:, m, :])
                nc.sync.dma_start(out=et16_sb[:, m, :], in_=et16_d[:, m, :])

            for cb in range(NCHUNK):
                bs = bass.ts(cb, CHUNK)
                for q, (i, m, r, idx, kbase, ksize) in enumerate(QUAD_META):
                    g = q % 32
                    tile_idx = q // 32
                    if g == 0:
                        o2_ps = opsum.tile([128, CHUNK], f32, tag="o2")

                    u_ps = upsum.tile([128, CHUNK], f32, tag="u")
                    nc.tensor.matmul(
                        u_ps[:],
                        wp_sb[kbase:kbase + ksize, idx, :],
                        et32_sb[kbase:kbase + ksize, i // 4, bs],
                        start=True,
                        stop=True,
                    )

                    t_sb = work.tile([128, CHUNK], f16, tag="t")
                    if q % PATH_B_EVERY == PATH_B_EVERY - 1:
                        # direct fp32 PSUM multiply on DVE (1x) - offloads ACT
                        nc.vector.tensor_mul(t_sb[:], u_ps[:], et16_sb[:, m, bs])
                    else:
                        u16 = work.tile([128, CHUNK], f16, tag="u16")
                        nc.scalar.copy(out=u16[:], in_=u_ps[:])
                        nc.vector.tensor_mul(t_sb[:], u16[:], et16_sb[:, m, bs])

                    last = (g == 31) or (q == NQ - 1)
                    nc.tensor.matmul(
                        o2_ps[:],
                        ind_sb[:, g, :],
                        t_sb[:],
                        start=(g == 0),
                        stop=last,
                    )
                    if last:
                        o2_sb = outs.tile([128, CHUNK], f32, tag="o2sb")
                        if tile_idx % 2 == 0:
                            nc.vector.tensor_copy(o2_sb[:], o2_ps[:])
                        else:
                            nc.scalar.copy(out=o2_sb[:], in_=o2_ps[:])
                        nc.sync.dma_start(out=o_d[tile_idx, :, cb, :], in_=o2_sb[:])

    nc.compile()
    _CACHED = nc
    return nc


# ---------------- public entry ----------------
def _run(embeddings, W, **spmd_kwargs):
    embeddings = np.ascontiguousarray(np.asarray(embeddings, dtype=np.float32))
    W = np.ascontiguousarray(np.asarray(W, dtype=np.float32))

    et32 = _pack_et(embeddings)                   # (8, 128, 10, 1024) f32
    et16 = et32.astype(np.float16)
    wp = _pack_w(W)
    ind = _make_ind()

    nc = _build()
    in_maps = [
        {"et32": et32[c], "et16": et16[c], "wp": wp, "ind": ind}
        for c in range(NCORES)
    ]
    res = run_bass_kernel_spmd(nc, in_maps, list(range(NCORES)), **spmd_kwargs)

    out = np.empty((BATCH, NPAIR), np.float32)
    for c in range(NCORES):
        o = res.results[c]["o"]                   # (NT, 128, 2, 512)
        o_flat = o.reshape(NT * 128, B_CORE)
        out[c * B_CORE:(c + 1) * B_CORE] = o_flat[SLOT_OF_K, :].T
    return out, res


def kernel(embeddings, W):
    out, _ = _run(embeddings, W)
    return out


# revision 7
# speedup vs baseline: 1.1782x; 1.1782x over previous
"""TRN2 Bass kernel for nn_BilinearInteraction.

Math: out[b,k] = sum_{e,f} E[b,i(k),e] * W[k,e,f] * E[b,j(k),f]
for the 780 upper-triangular field pairs (i<j) of 40 fields, embed dim 32.

Strategy (per core, batch-sharded 8 ways, B_CORE=1024):
- Host packs transposed embeddings ET[128=(g%4)*32+e, g//4, b] (f32 and f16
  copies) and W into "quads": quad (i, m) covers pairs (i, 4m+c), c=0..3
  (j-block m is 4-field aligned so the fp16 multiplier is one tile slice).
- Stage 1 (PE, fp16): u[(c,f), b] = Wq.T @ ET_i  (128x512)
- Evict PSUM->SBUF fp16 (ScalarE) then multiply by ET16 block m (VectorE,
  2x mode); every 4th quad multiplies straight from PSUM (fp32, 1x) to
  offload ScalarE.
- Stage 2 (PE, fp16): indicator matmul folds f-groups: out2[4g+c, b] +=
  sum_f t[(c,f), b]; 32 quads accumulate into one PSUM tile.
- Evict out2 to SBUF, DMA to DRAM; host gathers slots -> (8192, 780).
"""

import numpy as np

import concourse.bass as bass
import concourse.mybir as mybir
import concourse.tile as tile
from concourse import bacc
from concourse.bass_utils import run_bass_kernel_spmd

# ---------------- problem constants (hardcoded) ----------------
NF = 40          # fields
E = 32           # embed dim
NPAIR = 780      # NF*(NF-1)/2
NB = NF // 4     # 10 aligned j-blocks
BATCH = 8192
NCORES = 8
B_CORE = BATCH // NCORES   # 1024
CHUNK = 512
NCHUNK = B_CORE // CHUNK   # 2
PATH_B_EVERY = 4           # every 4th quad: DVE multiplies direct from PSUM

# ---------------- quad tables (pure python, deterministic) ----------------
_quads = []
for _i in range(NF):
    for _m in range(NB):
        if 4 * _m + 3 > _i:          # block m has some j > i
            _quads.append((_i, _m))
NQ = len(_quads)                      # 210
NT = (4 * NQ + 127) // 128            # 7 out2 tiles of 128 slots

_pair2k = {}
_k = 0
for _i in range(NF):
    for _j in range(_i + 1, NF):
        _pair2k[(_i, _j)] = _k
        _k += 1

# Matmul operand base partitions must be in {0, 32, 64}. Residues 0-2 use
# K=32 at base 32*r and share W columns 0..54; residue 3 uses K=64 at base
# 64 (rows 64:96 zeroed, W at 96:128) in its own column range 55..99.
_res_count = [0, 0, 0, 0]
QUAD_META = []                        # (i, m, r, idx, kbase, ksize)
for _i, _m in _quads:
    _r = _i % 4
    _idx = _res_count[_r]
    _res_count[_r] += 1
    if _r < 3:
        QUAD_META.append((_i, _m, _r, _idx, 32 * _r, 32))
    else:
        QUAD_META.append((_i, _m, _r, 55 + _idx, 64, 64))
WL = max(_res_count[:3]) + _res_count[3]   # 55 + 45 = 100

SLOT_OF_K = np.full(NPAIR, -1, np.int64)
for _q, (_i, _m) in enumerate(_quads):
    for _c in range(4):
        _j = 4 * _m + _c
        if _j > _i:
            SLOT_OF_K[_pair2k[(_i, _j)]] = 4 * _q + _c
assert (SLOT_OF_K >= 0).all()


# ---------------- host packing ----------------
def _pack_w(W):
    Wp = np.zeros((128, WL, 128), np.float32)
    for (i, m, r, idx, kbase, ksize) in QUAD_META:
        for c in range(4):
            j = 4 * m + c
            if j > i:
                Wp[32 * r:32 * r + 32, idx, 32 * c:32 * c + 32] = W[_pair2k[(i, j)]]
    return Wp


def _pack_et(emb):
    # emb (8192, 40, 32) -> (8 cores, 128, 10, 1024); partition = (g%4)*32+e
    et = emb.reshape(NCORES, B_CORE, NB, 4, E).transpose(0, 3, 4, 2, 1)
    return np.ascontiguousarray(et.reshape(NCORES, 128, NB, B_CORE))


def _make_ind():
    ind = np.zeros((128, 32, 128), np.float16)
    for g in range(32):
        for c in range(4):
            for f in range(E):
                ind[32 * c + f, g, 4 * g + c] = 1.0
    return ind


# ---------------- bass program ----------------
_CACHED = None


def _build():
    global _CACHED
    if _CACHED is not None:
        return _CACHED

    nc = bacc.Bacc("TRN2", target_bir_lowering=False, debug=False)
    f32r = mybir.dt.float32r
    f32 = mybir.dt.float32
    f16 = mybir.dt.float16

    et16_d = nc.dram_tensor("et16", [128, NB, B_CORE], f16, kind="ExternalInput")
    wp_d = nc.dram_tensor("wp", [128, WL, 128], f16, kind="ExternalInput")
    ind_d = nc.dram_tensor("ind", [128, 32, 128], f16, kind="ExternalInput")
    o_d = nc.dram_tensor("o", [NT, 128, NCHUNK, CHUNK], f32, kind="ExternalOutput")

    with tile.TileContext(nc) as tc:
        with (
            tc.tile_pool(name="consts", bufs=1) as consts,
            tc.tile_pool(name="work", bufs=4) as work,
            tc.tile_pool(name="outs", bufs=4) as outs,
            tc.tile_pool(name="upsum", bufs=4, space="PSUM") as upsum,
            tc.tile_pool(name="opsum", bufs=2, space="PSUM") as opsum,
        ):
            wp_sb = consts.tile([128, WL, 128], f16)
            nc.sync.dma_start(out=wp_sb[:], in_=wp_d[:])
            ind_sb = consts.tile([128, 32, 128], f16)
            nc.sync.dma_start(out=ind_sb[:], in_=ind_d[:])
            et16_sb = consts.tile([128, NB, B_CORE], f16)
            for m in range(NB):
                nc.sync.dma_start(out=et16_sb[:, m, :], in_=et16_d[:, m, :])

            for cb in range(NCHUNK):
                bs = bass.ts(cb, CHUNK)
                for q, (i, m, r, idx, kbase, ksize) in enumerate(QUAD_META):
                    g = q % 32
                    tile_idx = q // 32
                    if g == 0:
                        o2_ps = opsum.tile([128, CHUNK], f32, tag="o2")

                    u_ps = upsum.tile([128, CHUNK], f32, tag="u")
                    nc.tensor.matmul(
                        u_ps[:],
                        wp_sb[kbase:kbase + ksize, idx, :],
                        et16_sb[kbase:kbase + ksize, i // 4, bs],
                        start=True,
                        stop=True,
                    )

                    t_sb = work.tile([128, CHUNK], f16, tag="t")
                    if q % PATH_B_EVERY == PATH_B_EVERY - 1:
                        # direct fp32 PSUM multiply on DVE (1x) - offloads ACT
                        nc.vector.tensor_mul(t_sb[:], u_ps[:], et16_sb[:, m, bs])
                    else:
                        u16 = work.tile([128, CHUNK], f16, tag="u16")
                        nc.scalar.copy(out=u16[:], in_=u_ps[:])
                        nc.vector.tensor_mul(t_sb[:], u16[:], et16_sb[:, m, bs])

                    last = (g == 31) or (q == NQ - 1)
                    nc.tensor.matmul(
                        o2_ps[:],
                        ind_sb[:, g, :],
                        t_sb[:],
                        start=(g == 0),
                        stop=last,
                    )
                    if last:
                        o2_sb = outs.tile([128, CHUNK], f32, tag="o2sb")
                        if tile_idx % 2 == 0:
                            nc.vector.tensor_copy(o2_sb[:], o2_ps[:])
                        else:
                            nc.scalar.copy(out=o2_sb[:], in_=o2_ps[:])
                        nc.sync.dma_start(out=o_d[tile_idx, :, cb, :], in_=o2_sb[:])

    nc.compile()
    _CACHED = nc
    return nc


# ---------------- public entry ----------------
def _run(embeddings, W, **spmd_kwargs):
    embeddings = np.ascontiguousarray(np.asarray(embeddings, dtype=np.float32))
    W = np.ascontiguousarray(np.asarray(W, dtype=np.float32))

    et16 = _pack_et(embeddings).astype(np.float16)  # (8, 128, 10, 1024)
    wp = _pack_w(W).astype(np.float16)
    ind = _make_ind()

    nc = _build()
    in_maps = [
        {"et16": et16[c], "wp": wp, "ind": ind}
        for c in range(NCORES)
    ]
    res = run_bass_kernel_spmd(nc, in_maps, list(range(NCORES)), **spmd_kwargs)

    out = np.empty((BATCH, NPAIR), np.float32)
    for c in range(NCORES):
        o = res.results[c]["o"]                   # (NT, 128, 2, 512)
        o_flat = o.reshape(NT * 128, B_CORE)
        out[c * B_CORE:(c + 1) * B_CORE] = o_flat[SLOT_OF_K, :].T
    return out, res


def kernel(embeddings, W):
    out, _ = _run(embeddings, W)
    return out


# revision 8
# speedup vs baseline: 1.1787x; 1.0004x over previous
"""TRN2 Bass kernel for nn_BilinearInteraction.

Math: out[b,k] = sum_{e,f} E[b,i(k),e] * W[k,e,f] * E[b,j(k),f]
for the 780 upper-triangular field pairs (i<j) of 40 fields, embed dim 32.

Strategy (per core, batch-sharded 8 ways, B_CORE=1024):
- Host packs transposed embeddings ET[128=(g%4)*32+e, g//4, b] (f32 and f16
  copies) and W into "quads": quad (i, m) covers pairs (i, 4m+c), c=0..3
  (j-block m is 4-field aligned so the fp16 multiplier is one tile slice).
- Stage 1 (PE, fp16): u[(c,f), b] = Wq.T @ ET_i  (128x512)
- Evict PSUM->SBUF fp16 (ScalarE) then multiply by ET16 block m (VectorE,
  2x mode); every 4th quad multiplies straight from PSUM (fp32, 1x) to
  offload ScalarE.
- Stage 2 (PE, fp16): indicator matmul folds f-groups: out2[4g+c, b] +=
  sum_f t[(c,f), b]; 32 quads accumulate into one PSUM tile.
- Evict out2 to SBUF, DMA to DRAM; host gathers slots -> (8192, 780).
"""

import numpy as np

import concourse.bass as bass
import concourse.mybir as mybir
import concourse.tile as tile
from concourse import bacc
from concourse.bass_utils import run_bass_kernel_spmd

# ---------------- problem constants (hardcoded) ----------------
NF = 40          # fields
E = 32           # embed dim
NPAIR = 780      # NF*(NF-1)/2
NB = NF // 4     # 10 aligned j-blocks
BATCH = 8192
NCORES = 8
B_CORE = BATCH // NCORES   # 1024
CHUNK = 512
NCHUNK = B_CORE // CHUNK   # 2
PATH_B_EVERY = 4           # every 4th quad: DVE multiplies direct from PSUM

# ---------------- quad tables (pure python, deterministic) ----------------
_quads = []
for _i in range(NF):
    for _m in range(NB):
        if 4 * _m + 3 > _i:          # block m has some j > i
            _quads.append((_i, _m))
NQ = len(_quads)                      # 210
NT = (4 * NQ + 127) // 128            # 7 out2 tiles of 128 slots

_pair2k = {}
_k = 0
for _i in range(NF):
    for _j in range(_i + 1, NF):
        _pair2k[(_i, _j)] = _k
        _k += 1

# Matmul operand base partitions must be in {0, 32, 64}. Residues 0-2 use
# K=32 at base 32*r and share W columns 0..54; residue 3 uses K=64 at base
# 64 (rows 64:96 zeroed, W at 96:128) in its own column range 55..99.
_res_count = [0, 0, 0, 0]
QUAD_META = []                        # (i, m, r, idx, kbase, ksize)
for _i, _m in _quads:
    _r = _i % 4
    _idx = _res_count[_r]
    _res_count[_r] += 1
    if _r < 3:
        QUAD_META.append((_i, _m, _r, _idx, 32 * _r, 32))
    else:
        QUAD_META.append((_i, _m, _r, 55 + _idx, 64, 64))
WL = max(_res_count[:3]) + _res_count[3]   # 55 + 45 = 100

SLOT_OF_K = np.full(NPAIR, -1, np.int64)
for _q, (_i, _m) in enumerate(_quads):
    for _c in range(4):
        _j = 4 * _m + _c
        if _j > _i:
            SLOT_OF_K[_pair2k[(_i, _j)]] = 4 * _q + _c
assert (SLOT_OF_K >= 0).all()


# ---------------- host packing ----------------
def _pack_w(W):
    Wp = np.zeros((128, WL, 128), np.float32)
    for (i, m, r, idx, kbase, ksize) in QUAD_META:
        for c in range(4):
            j = 4 * m + c
            if j > i:
                Wp[32 * r:32 * r + 32, idx, 32 * c:32 * c + 32] = W[_pair2k[(i, j)]]
    return Wp


def _pack_et(emb):
    # emb (8192, 40, 32) -> (8 cores, 128, 10, 1024); partition = (g%4)*32+e
    et = emb.reshape(NCORES, B_CORE, NB, 4, E).transpose(0, 3, 4, 2, 1)
    return np.ascontiguousarray(et.reshape(NCORES, 128, NB, B_CORE))


def _make_ind():
    ind = np.zeros((128, 32, 128), np.float16)
    for g in range(32):
        for c in range(4):
            for f in range(E):
                ind[32 * c + f, g, 4 * g + c] = 1.0
    return ind


# ---------------- bass program ----------------
_CACHED = None


def _build():
    global _CACHED
    if _CACHED is not None:
        return _CACHED

    nc = bacc.Bacc("TRN2", target_bir_lowering=False, debug=False)
    f32r = mybir.dt.float32r
    f32 = mybir.dt.float32
    f16 = mybir.dt.float16

    et16_d = nc.dram_tensor("et16", [128, NB, B_CORE], f16, kind="ExternalInput")
    wp_d = nc.dram_tensor("wp", [128, WL, 128], f16, kind="ExternalInput")
    ind_d = nc.dram_tensor("ind", [128, 32, 128], f16, kind="ExternalInput")
    o_d = nc.dram_tensor("o", [NT, 128, NCHUNK, CHUNK], f32, kind="ExternalOutput")

    with tile.TileContext(nc) as tc:
        with (
            tc.tile_pool(name="consts", bufs=1) as consts,
            tc.tile_pool(name="work", bufs=8) as work,
            tc.tile_pool(name="outs", bufs=4) as outs,
            tc.tile_pool(name="upsum", bufs=6, space="PSUM") as upsum,
            tc.tile_pool(name="opsum", bufs=2, space="PSUM") as opsum,
        ):
            wp_sb = consts.tile([128, WL, 128], f16)
            nc.sync.dma_start(out=wp_sb[:], in_=wp_d[:])
            ind_sb = consts.tile([128, 32, 128], f16)
            nc.sync.dma_start(out=ind_sb[:], in_=ind_d[:])
            et16_sb = consts.tile([128, NB, B_CORE], f16)
            for m in range(NB):
                nc.sync.dma_start(out=et16_sb[:, m, :], in_=et16_d[:, m, :])

            for cb in range(NCHUNK):
                bs = bass.ts(cb, CHUNK)
                for q, (i, m, r, idx, kbase, ksize) in enumerate(QUAD_META):
                    g = q % 32
                    tile_idx = q // 32
                    if g == 0:
                        o2_ps = opsum.tile([128, CHUNK], f32, tag="o2")

                    u_ps = upsum.tile([128, CHUNK], f32, tag="u")
                    nc.tensor.matmul(
                        u_ps[:],
                        wp_sb[kbase:kbase + ksize, idx, :],
                        et16_sb[kbase:kbase + ksize, i // 4, bs],
                        start=True,
                        stop=True,
                    )

                    t_sb = work.tile([128, CHUNK], f16, tag="t")
                    if q % PATH_B_EVERY == PATH_B_EVERY - 1:
                        # direct fp32 PSUM multiply on DVE (1x) - offloads ACT
                        nc.vector.tensor_mul(t_sb[:], u_ps[:], et16_sb[:, m, bs])
                    else:
                        u16 = work.tile([128, CHUNK], f16, tag="u16")
                        nc.scalar.copy(out=u16[:], in_=u_ps[:])
                        nc.vector.tensor_mul(t_sb[:], u16[:], et16_sb[:, m, bs])

                    last = (g == 31) or (q == NQ - 1)
                    nc.tensor.matmul(
                        o2_ps[:],
                        ind_sb[:, g, :],
                        t_sb[:],
                        start=(g == 0),
                        stop=last,
                    )
                    if last:
                        o2_sb = outs.tile([128, CHUNK], f32, tag="o2sb")
                        if tile_idx % 2 == 0:
                            nc.vector.tensor_copy(o2_sb[:], o2_ps[:])
                        else:
                            nc.scalar.copy(out=o2_sb[:], in_=o2_ps[:])
                        nc.sync.dma_start(out=o_d[tile_idx, :, cb, :], in_=o2_sb[:])

    nc.compile()
    _CACHED = nc
    return nc


# ---------------- public entry ----------------
def _run(embeddings, W, **spmd_kwargs):
    embeddings = np.ascontiguousarray(np.asarray(embeddings, dtype=np.float32))
    W = np.ascontiguousarray(np.asarray(W, dtype=np.float32))

    et16 = _pack_et(embeddings).astype(np.float16)  # (8, 128, 10, 1024)
    wp = _pack_w(W).astype(np.float16)
    ind = _make_ind()

    nc = _build()
    in_maps = [
        {"et16": et16[c], "wp": wp, "ind": ind}
        for c in range(NCORES)
    ]
    res = run_bass_kernel_spmd(nc, in_maps, list(range(NCORES)), **spmd_kwargs)

    out = np.empty((BATCH, NPAIR), np.float32)
    for c in range(NCORES):
        o = res.results[c]["o"]                   # (NT, 128, 2, 512)
        o_flat = o.reshape(NT * 128, B_CORE)
        out[c * B_CORE:(c + 1) * B_CORE] = o_flat[SLOT_OF_K, :].T
    return out, res


def kernel(embeddings, W):
    out, _ = _run(embeddings, W)
    return out


# revision 13
# speedup vs baseline: 1.4891x; 1.2634x over previous
"""TRN2 Bass kernel for nn_BilinearInteraction.

Math: out[b,k] = sum_{e,f} E[b,i(k),e] * W[k,e,f] * E[b,j(k),f]
for the 780 upper-triangular field pairs (i<j) of 40 fields, embed dim 32.

Strategy (per core, batch-sharded 8 ways, B_CORE=1024):
- Host packs transposed embeddings ET16[128=(g%4)*32+e, g//4, b] (fp16)
  and W into "quads": quad (i, m) covers pairs (i, 4m+c), c=0..3
  (j-block m is 4-field aligned so the fp16 multiplier is one tile slice).
- Stage 1 (PE, fp16): u[(c,f), b] = Wq.T @ ET_i  (128x512)
- Evict PSUM->SBUF fp16 (ScalarE) then multiply by ET16 block m (VectorE,
  2x mode); every 4th quad multiplies straight from PSUM (fp32, 1x) to
  offload ScalarE.
- Stage 2 (PE, fp16): indicator matmul folds f-groups: out2[4g+c, b] +=
  sum_f t[(c,f), b]; 32 quads accumulate into one PSUM tile.
- Evict out2 to SBUF, DMA to DRAM; host gathers slots -> (8192, 780).
"""

import numpy as np

import concourse.bass as bass
import concourse.mybir as mybir
import concourse.tile as tile
from concourse import bacc
from concourse.bass_utils import run_bass_kernel_spmd

# ---------------- problem constants (hardcoded) ----------------
NF = 40          # fields
E = 32           # embed dim
NPAIR = 780      # NF*(NF-1)/2
NB = NF // 4     # 10 aligned j-blocks
BATCH = 8192
NCORES = 8
B_CORE = BATCH // NCORES   # 1024
CHUNK = 512
NCHUNK = B_CORE // CHUNK   # 2
PATH_B_EVERY = 4           # every 4th quad: DVE multiplies direct from PSUM

# ---------------- quad tables (pure python, deterministic) ----------------
_quads = []
for _i in range(NF):
    for _m in range(NB):
        if 4 * _m + 3 > _i:          # block m has some j > i
            _quads.append((_i, _m))
NQ = len(_quads)                      # 210
NT = (4 * NQ + 127) // 128            # 7 out2 tiles of 128 slots

_pair2k = {}
_k = 0
for _i in range(NF):
    for _j in range(_i + 1, NF):
        _pair2k[(_i, _j)] = _k
        _k += 1

# Matmul operand base partitions must be in {0, 32, 64}. Residues 0-2 use
# K=32 at base 32*r and share W columns 0..54; residue 3 uses K=64 at base
# 64 (rows 64:96 zeroed, W at 96:128) in its own column range 55..99.
_res_count = [0, 0, 0, 0]
QUAD_META = []                        # (i, m, r, idx, kbase, ksize)
for _i, _m in _quads:
    _r = _i % 4
    _idx = _res_count[_r]
    _res_count[_r] += 1
    if _r < 3:
        QUAD_META.append((_i, _m, _r, _idx, 32 * _r, 32))
    else:
        QUAD_META.append((_i, _m, _r, 55 + _idx, 64, 64))
WL = max(_res_count[:3]) + _res_count[3]   # 55 + 45 = 100

SLOT_OF_K = np.full(NPAIR, -1, np.int64)
for _q, (_i, _m) in enumerate(_quads):
    for _c in range(4):
        _j = 4 * _m + _c
        if _j > _i:
            SLOT_OF_K[_pair2k[(_i, _j)]] = 4 * _q + _c
assert (SLOT_OF_K >= 0).all()


# ---------------- host packing ----------------
def _pack_w(W):
    Wp = np.zeros((128, WL, 128), np.float32)
    for (i, m, r, idx, kbase, ksize) in QUAD_META:
        for c in range(4):
            j = 4 * m + c
            if j > i:
                Wp[32 * r:32 * r + 32, idx, 32 * c:32 * c + 32] = W[_pair2k[(i, j)]]
    return Wp


def _pack_et(emb):
    # emb (8192, 40, 32) -> (8 cores, 128, 10, 1024); partition = (g%4)*32+e
    et = emb.reshape(NCORES, B_CORE, NB, 4, E).transpose(0, 3, 4, 2, 1)
    return np.ascontiguousarray(et.reshape(NCORES, 128, NB, B_CORE))


def _make_ind():
    ind = np.zeros((128, 32, 128), np.float16)
    for g in range(32):
        for c in range(4):
            for f in range(E):
                ind[32 * c + f, g, 4 * g + c] = 1.0
    return ind


# ---------------- bass program ----------------
_CACHED = None


def _build():
    global _CACHED
    if _CACHED is not None:
        return _CACHED

    nc = bacc.Bacc("TRN2", target_bir_lowering=False, debug=False)
    f32 = mybir.dt.float32
    f16 = mybir.dt.float16

    et16_d = nc.dram_tensor("et16", [128, NB, B_CORE], f16, kind="ExternalInput")
    wp_d = nc.dram_tensor("wp", [128, WL, 128], f16, kind="ExternalInput")
    ind_d = nc.dram_tensor("ind", [128, 32, 128], f16, kind="ExternalInput")
    o_d = nc.dram_tensor("o", [NT, 128, NCHUNK, CHUNK], f32, kind="ExternalOutput")

    with tile.TileContext(nc) as tc:
        with (
            tc.tile_pool(name="consts", bufs=1) as consts,
            tc.tile_pool(name="work", bufs=8) as work,
            tc.tile_pool(name="outs", bufs=4) as outs,
            tc.tile_pool(name="upsum", bufs=6, space="PSUM") as upsum,
            tc.tile_pool(name="opsum", bufs=2, space="PSUM") as opsum,
        ):
            ind_sb = consts.tile([128, 32, 128], f16)
            nc.sync.dma_start(out=ind_sb[:], in_=ind_d[:])
            wp_sb = consts.tile([128, WL, 128], f16)
            for s in range(0, WL, 25):
                e = min(s + 25, WL)
                nc.sync.dma_start(out=wp_sb[:, s:e, :], in_=wp_d[:, s:e, :])
            et16_sb = consts.tile([128, NB, B_CORE], f16)
            for m in range(NB):
                nc.sync.dma_start(out=et16_sb[:, m, :], in_=et16_d[:, m, :])

            for cb in range(NCHUNK):
                bs = bass.ts(cb, CHUNK)
                for q, (i, m, r, idx, kbase, ksize) in enumerate(QUAD_META):
                    g = q % 32
                    tile_idx = q // 32
                    if g == 0:
                        o2_ps = opsum.tile([128, CHUNK], f32, tag="o2")

                    u_ps = upsum.tile([128, CHUNK], f32, tag="u")
                    nc.tensor.matmul(
                        u_ps[:],
                        wp_sb[kbase:kbase + ksize, idx, :],
                        et16_sb[kbase:kbase + ksize, i // 4, bs],
                        start=True,
                        stop=True,
                    )

                    t_sb = work.tile([128, CHUNK], f16, tag="t")
                    if q % PATH_B_EVERY == PATH_B_EVERY - 1:
                        # direct fp32 PSUM multiply on DVE (1x) - offloads ACT
                        nc.vector.tensor_mul(t_sb[:], u_ps[:], et16_sb[:, m, bs])
                    else:
                        u16 = work.tile([128, CHUNK], f16, tag="u16")
                        nc.scalar.copy(out=u16[:], in_=u_ps[:])
                        nc.vector.tensor_mul(t_sb[:], u16[:], et16_sb[:, m, bs])

                    last = (g == 31) or (q == NQ - 1)
                    nc.tensor.matmul(
                        o2_ps[:],
                        ind_sb[:, g, :],
                        t_sb[:],
                        start=(g == 0),
                        stop=last,
                    )
                    if last:
                        o2_sb = outs.tile([128, CHUNK], f32, tag="o2sb")
                        if tile_idx % 2 == 0:
                            nc.vector.tensor_copy(o2_sb[:], o2_ps[:])
                        else:
                            nc.scalar.copy(out=o2_sb[:], in_=o2_ps[:])
                        nc.sync.dma_start(out=o_d[tile_idx, :, cb, :], in_=o2_sb[:])

    nc.compile()
    _CACHED = nc
    return nc


# ---------------- public entry ----------------
def _run(embeddings, W, **spmd_kwargs):
    embeddings = np.ascontiguousarray(np.asarray(embeddings, dtype=np.float32))
    W = np.ascontiguousarray(np.asarray(W, dtype=np.float32))

    et16 = _pack_et(embeddings).astype(np.float16)  # (8, 128, 10, 1024)
    wp = _pack_w(W).astype(np.float16)
    ind = _make_ind()

    nc = _build()
    in_maps = [
        {"et16": et16[c], "wp": wp, "ind": ind}
        for c in range(NCORES)
    ]
    res = run_bass_kernel_spmd(nc, in_maps, list(range(NCORES)), **spmd_kwargs)

    out = np.empty((BATCH, NPAIR), np.float32)
    for c in range(NCORES):
        o = res.results[c]["o"]                   # (NT, 128, 2, 512)
        o_flat = o.reshape(NT * 128, B_CORE)
        out[c * B_CORE:(c + 1) * B_CORE] = o_flat[SLOT_OF_K, :].T
    return out, res


def kernel(embeddings, W):
    out, _ = _run(embeddings, W)
    return out
